# revision 1
# baseline (speedup 1.0000x reference)
"""LBLHighwayBiLm Trainium2 kernel (8-core data-parallel over batch).

v2: fp16 activations/weights everywhere (rel_err ~6e-3 vs 2e-2 gate).
Highway matmuls on PE (fp16, K=256 as 2 accumulating 128-matmuls, psum
tiles [128,1024] so evictions run 1024-wide). The 5-tap conv is split
across three backends per (layer,dir,blk) group: "pe" (4 taps as diagonal
matmuls accumulated in PSUM + tap0 folded into the DVE STT eviction),
"dve" (5 tensor_scalar mults + 4 tensor_tensor adds), and "dma" (tap0
tensor_scalar + 4 SWDGE accumulate-DMAs). Sigmoid evictions on ACT; relu
evictions split ACT/DVE per a pattern; highway combine sub/mult on DVE;
the final add is a SWDGE accumulate-DMA into the destination buffer
(which the relu eviction pre-filled with r). Each core: 4 batch rows.
"""

import numpy as np

import concourse.bacc as bacc
import concourse.tile as tile
import concourse.mybir as mybir
from concourse.bass_utils import run_bass_kernel_spmd

F16 = mybir.dt.float16
F32 = mybir.dt.float32
AOP = mybir.AluOpType
AFT = mybir.ActivationFunctionType

N_LAYERS = 2
N_HW = 2
W = 4
D = 256
B, S = 32, 1024
NCORES = 8
BLOC = B // NCORES          # 4 batch rows per core
T = BLOC * S                # 4096 tokens per core
PB = D // 128               # 2 partition blocks for D
EB = (2 * D) // 128         # 4 partition blocks for 2D
ROW0 = S + 2 * W            # layer-0 padded row: 1032
ROW1 = S + W                # layer-1 padded row: 1028
CH = 1024                   # token chunk = one batch row

# ---- schedule knobs ---------------------------------------------------------
INTERLEAVE_DIRS = 0
TAIL_FINE = 1
ADD_H0 = ""         # override ADD_MODE for h==0 sublayers
ADD_TAIL_V = 1      # final sublayer add on DVE (in-place), skip DMA hop
DEFER_L0_OUT = 0    # emit layer-0 output DMAs after layer-1 code
# conv backend per (l, d, blk) group (index = l*4 + d*2 + blk)
CONV_PAT = ["pe", "dve", "dma", "pe", "pea", "dma", "pea", "dve"]
CONV_HIPRI = True
# relu eviction engine stream: "a"=ACT, "v"=DVE tensor_scalar, "p"=Pool
RELU_PAT = ["a", "a", "v", "a"]
# final highway add: "dma" = SWDGE accumulate, "v" = DVE tensor_tensor
ADD_MODE = "dma"
ADD_SPLIT = 2               # accum-DMAs per (l,d,h,blk): 1, 2 or 4
ADD_SPLIT_H0 = 2            # split for h==0 sublayers (hwB start lag)
CONV_DMA_ROWS = 2           # rows per conv-dma accumulate segment
MULT_PAT = ["p", "v", "v", "v"]
SUB_PAT = ["v"]
OUT_ROWS = 2                # rows per output DMA
HW_PSUM_BUFS = 3            # [128,1024] f32 tiles (2 banks each)
CV_PSUM_BUFS = 1            # conv psum tiles
CV_W = 1024                 # conv psum width (512 or 1024)
SCRATCH_BUFS = 2
G_BUFS = 2
T_BUFS = 2
SPLIT_LOAD = 1      # rows 2-3 via HWDGE f32 + ACT cast (startup overlap)
EVICT_HIPRI = 0
ADD_HEAD_V = 0      # first segment of h0 adds on DVE (skip DMA latency)
HW0_HIPRI = 0       # prioritize the very first highway token-group
CONV_ROW0_PE = 0    # layer-0 dve-conv groups: do row 0 on PE (worse: cv-slot WAR)


def _eng(nc, code):
    return {"v": nc.vector, "g": nc.gpsimd, "p": nc.gpsimd}[code]


def build_bass(params):
    nc = bacc.Bacc(target_bir_lowering=False)

    x_in = nc.dram_tensor("x", [PB, 128, BLOC * ROW0], F32, kind="ExternalInput")
    out = nc.dram_tensor("out", [N_LAYERS, 2, PB, 128, T], F16,
                         kind="ExternalOutput")

    wt_dram = nc.inline_tensor(params["wt"], name="wt")        # [L,2,HW,PB,128,512] f16
    bias_dram = nc.inline_tensor(params["bias"], name="bias")  # [128, L*2*HW*EB] f32
    pad_dram = nc.inline_tensor(params["pad1"], name="pad1")   # [128, 2*PB*W] f16
    diag_dram = nc.inline_tensor(params["diag"], name="diag")  # [L,2,128,5*128] f16
    fw = params["fwd_w"]
    bw = params["bwd_w"]

    relu_ctr = [0]
    mult_ctr = [0]

    with tile.TileContext(nc) as tc:
        consts = tc.alloc_tile_pool(name="consts", bufs=1)
        bufs = tc.alloc_tile_pool(name="bufs", bufs=1)
        scratch = tc.alloc_tile_pool(name="scratch", bufs=SCRATCH_BUFS)
        psum_hw = tc.alloc_tile_pool(name="psum_hw", bufs=HW_PSUM_BUFS,
                                     space="PSUM")
        psum_cv = tc.alloc_tile_pool(name="psum_cv", bufs=CV_PSUM_BUFS,
                                     space="PSUM")

        # ---- constants (diag/bias/pad first: the first PE work needs diag,
        # highway weights are not needed until ~14us in) -----------------------
        diag_sb = {}
        for l in range(N_LAYERS):
            for di in range(2):
                t_ = consts.tile([128, 5 * 128], F16, tag=f"dg{l}{di}",
                                 name=f"dg{l}{di}")
                nc.sync.dma_start(out=t_, in_=diag_dram[l, di])
                diag_sb[(l, di)] = t_
        bias_sb = consts.tile([128, N_LAYERS * 2 * N_HW * EB], F32,
                              name="bias_sb")
        nc.sync.dma_start(out=bias_sb, in_=bias_dram[:, :])
        pad_sb = consts.tile([128, 2 * PB * W], F16, name="pad_sb")
        nc.sync.dma_start(out=pad_sb, in_=pad_dram[:, :])
        wt_sb = {}
        for l in range(N_LAYERS):
            for di in range(2):
                for h in range(N_HW):
                    for kb in range(PB):
                        t_ = consts.tile([128, 2 * D], F16,
                                         tag=f"wt{l}{di}{h}{kb}",
                                         name=f"wt{l}{di}{h}{kb}")
                        nc.sync.dma_start(out=t_, in_=wt_dram[l, di, h, kb])
                        wt_sb[(l, di, h, kb)] = t_

        def bias_ap(l, di, h, eb):
            i = ((l * 2 + di) * N_HW + h) * EB + eb
            return bias_sb[:, i:i + 1]

        # ---- layer-0 padded input: one casting DMA per block ---------------
        xpad0 = [bufs.tile([128, BLOC * ROW0], F16, tag=f"xp0_{b}",
                           name=f"xp0_{b}") for b in range(PB)]
        for r in range(BLOC):
            for blk in range(PB):
                if SPLIT_LOAD and r >= 2:
                    stg = scratch.tile([128, ROW0], F32, tag="xstg",
                                       name=f"xstg{blk}{r}")
                    nc.sync.dma_start(
                        out=stg, in_=x_in[blk][:, r * ROW0:(r + 1) * ROW0])
                    nc.scalar.activation(
                        xpad0[blk][:, r * ROW0:(r + 1) * ROW0], stg, AFT.Copy)
                else:
                    nc.gpsimd.dma_start(
                        out=xpad0[blk][:, r * ROW0:(r + 1) * ROW0],
                        in_=x_in[blk][:, r * ROW0:(r + 1) * ROW0])

        # ---- conv -----------------------------------------------------------
        def conv(l, di, blk, dst, src, row_len, base_off, taps):
            """dst [128, T] <- 5-tap conv of src rows; mode per CONV_PAT."""
            mode = CONV_PAT[(l * 4 + di * 2 + blk) % len(CONV_PAT)]
            w0 = float(taps[0])
            if mode == "dve":
                for r in range(BLOC):
                    if CONV_ROW0_PE and l == 0 and r == 0:
                        dg = diag_sb[(l, di)]
                        o = base_off
                        ps = psum_cv.tile([128, CV_W], F32, tag="cv",
                                          name=f"cvz{l}{di}{blk}")
                        for half in range(CV_W // 512):
                            for j in range(1, 5):
                                nc.tensor.matmul(
                                    ps[:, half * 512:(half + 1) * 512],
                                    lhsT=dg[:, j * 128:(j + 1) * 128],
                                    rhs=src[:, o + half * 512 + j:
                                            o + half * 512 + j + 512],
                                    start=(j == 1), stop=(j == 4),
                                )
                        nc.vector.scalar_tensor_tensor(
                            dst[:, 0:CH], src[:, o:o + CH], w0, ps,
                            AOP.mult, AOP.add)
                        continue
                    def sl(j):
                        o = r * row_len + base_off + j
                        return src[:, o:o + CH]
                    sA = scratch.tile([128, CH], F16, tag="cmA",
                                      name=f"cmA{l}{di}{blk}{r}")
                    sB = scratch.tile([128, CH], F16, tag="cmB",
                                      name=f"cmB{l}{di}{blk}{r}")
                    nc.vector.tensor_scalar_mul(sA, sl(0), float(taps[0]))
                    for j in range(1, 4):
                        nc.vector.tensor_scalar_mul(sB, sl(j), float(taps[j]))
                        nc.vector.tensor_tensor(sA, sA, sB, AOP.add)
                    nc.vector.tensor_scalar_mul(sB, sl(4), float(taps[4]))
                    nc.vector.tensor_tensor(dst[:, r * CH:(r + 1) * CH],
                                            sA, sB, AOP.add)
            elif mode == "pea":
                dg = diag_sb[(l, di)]
                for r in range(BLOC):
                    o = r * row_len + base_off
                    for part in range(CH // CV_W):
                        ps = psum_cv.tile([128, CV_W], F32, tag="cv",
                                          name=f"cva{l}{di}{blk}{r}{part}")
                        po = o + part * CV_W
                        for half in range(CV_W // 512):
                            for j in range(5):
                                nc.tensor.matmul(
                                    ps[:, half * 512:(half + 1) * 512],
                                    lhsT=dg[:, j * 128:(j + 1) * 128],
                                    rhs=src[:, po + half * 512 + j:
                                            po + half * 512 + j + 512],
                                    start=(j == 0), stop=(j == 4),
                                )
                        nc.scalar.activation(
                            dst[:, r * CH + part * CV_W:
                                r * CH + (part + 1) * CV_W],
                            ps, AFT.Copy)
            elif mode == "pe":
                dg = diag_sb[(l, di)]
                for r in range(BLOC):
                    o = r * row_len + base_off
                    for part in range(CH // CV_W):
                        ps = psum_cv.tile([128, CV_W], F32, tag="cv",
                                          name=f"cv{l}{di}{blk}{r}{part}")
                        po = o + part * CV_W
                        for half in range(CV_W // 512):
                            for j in range(1, 5):
                                nc.tensor.matmul(
                                    ps[:, half * 512:(half + 1) * 512],
                                    lhsT=dg[:, j * 128:(j + 1) * 128],
                                    rhs=src[:, po + half * 512 + j:
                                            po + half * 512 + j + 512],
                                    start=(j == 1), stop=(j == 4),
                                )
                        nc.vector.scalar_tensor_tensor(
                            dst[:, r * CH + part * CV_W:
                                r * CH + (part + 1) * CV_W],
                            src[:, po:po + CV_W], w0, ps, AOP.mult, AOP.add)
        def conv_dma(l, di, blk, dst, src, row_len, base_off, taps):
            src3 = src.rearrange("p (r c) -> p r c", c=row_len)
            dst3 = dst.rearrange("p (r c) -> p r c", c=CH)
            rp = CONV_DMA_ROWS
            for s in range(BLOC // rp):
                r0, r1 = s * rp, (s + 1) * rp
                nc.vector.tensor_scalar_mul(
                    dst3[:, r0:r1], src3[:, r0:r1, base_off:base_off + CH],
                    float(taps[0]))
                for j in range(1, 5):
                    sc = scratch.tile([128, rp * CH], F16, tag="cs",
                                      name=f"cs{l}{di}{blk}{s}{j}")
                    sc3 = sc.rearrange("p (r c) -> p r c", c=CH)
                    nc.vector.tensor_scalar_mul(
                        sc3, src3[:, r0:r1, base_off + j:base_off + j + CH],
                        float(taps[j]))
                    nc.gpsimd.dma_start(out=dst3[:, r0:r1], in_=sc3,
                                        accum_op=AOP.add)

        # ---- one highway sublayer ------------------------------------------
        def highway_tg(l, di, h, tg, x0, x1, x1_row_len, x1_off, u_seg):
            """Emit one token-group (row) of a highway sublayer."""
            am = ADD_MODE if not (ADD_H0 and h == 0) else ADD_H0
            add_dma = (am == "dma")
            rows_per = BLOC // (ADD_SPLIT_H0 if h == 0 else ADD_SPLIT)
            if TAIL_FINE and l == N_LAYERS - 1 and h == 1:
                rows_per = 1
            if True:
                g_c = {}
                r_dst = {}
                for eb in range(EB):
                    ps = psum_hw.tile([128, CH], F32, tag="hw",
                                      name=f"hp{l}{di}{h}{tg}{eb}")
                    for half in range(2):
                        for kb in range(PB):
                            nc.tensor.matmul(
                                ps[:, half * 512:(half + 1) * 512],
                                lhsT=wt_sb[(l, di, h, kb)][:, eb * 128:(eb + 1) * 128],
                                rhs=x0[kb][:, tg * CH + half * 512:tg * CH + (half + 1) * 512],
                                start=(kb == 0), stop=(kb == PB - 1),
                            )
                    import contextlib
                    ehp = (tc.high_priority() if EVICT_HIPRI
                           else contextlib.nullcontext())
                    if eb >= PB:  # gate blocks
                        blk = eb - PB
                        gt = scratch.tile([128, CH], F16, tag=f"g{blk}",
                                          bufs=G_BUFS,
                                          name=f"g{l}{di}{h}{tg}{blk}")
                        with ehp:
                            nc.scalar.activation(gt, ps, AFT.Sigmoid,
                                                 bias=bias_ap(l, di, h, eb),
                                                 scale=1.0)
                        g_c[blk] = gt
                    else:         # nonlinear blocks -> relu evict
                        blk = eb
                        if add_dma:
                            o = tg * x1_row_len + x1_off
                            rd = x1[blk][:, o:o + CH]
                        else:
                            rd = scratch.tile([128, CH], F16, tag=f"r{blk}",
                                              name=f"r{l}{di}{h}{tg}{blk}")
                        eng = RELU_PAT[relu_ctr[0] % len(RELU_PAT)]
                        relu_ctr[0] += 1
                        with ehp:
                            if eng == "a":
                                nc.scalar.activation(rd, ps, AFT.Relu,
                                                     bias=bias_ap(l, di, h, eb),
                                                     scale=1.0)
                            else:
                                _eng(nc, eng).tensor_scalar(
                                    rd, ps, bias_ap(l, di, h, eb), 0.0,
                                    AOP.add, AOP.max)
                        r_dst[blk] = rd
                for blk in range(PB):
                    x0c = x0[blk][:, tg * CH:(tg + 1) * CH]
                    tt = scratch.tile([128, CH], F16, tag=f"t{blk}",
                                      bufs=T_BUFS,
                                      name=f"t{l}{di}{h}{tg}{blk}")
                    seng = SUB_PAT[mult_ctr[0] % len(SUB_PAT)]
                    _eng(nc, seng).tensor_tensor(tt, x0c, r_dst[blk],
                                                 AOP.subtract)
                    if add_dma:
                        seg = tg // rows_per
                        if (blk, seg) not in u_seg:
                            u_seg[(blk, seg)] = scratch.tile(
                                [128, rows_per * CH], F16, tag=f"u{di}{blk}",
                                name=f"u{l}{di}{h}{blk}{seg}")
                        uc = u_seg[(blk, seg)][:, (tg % rows_per) * CH:
                                               (tg % rows_per + 1) * CH]
                        meng = MULT_PAT[mult_ctr[0] % len(MULT_PAT)]
                        mult_ctr[0] += 1
                        _eng(nc, meng).tensor_tensor(uc, g_c[blk], tt,
                                                     AOP.mult)
                        if tg % rows_per == rows_per - 1:
                            tail_v = (ADD_TAIL_V and l == N_LAYERS - 1
                                      and h == 1)
                            x13 = x1[blk].rearrange("p (r c) -> p r c",
                                                    c=x1_row_len)
                            r0 = seg * rows_per
                            if tail_v:
                                dst = x1[blk][:, tg * x1_row_len + x1_off:
                                              tg * x1_row_len + x1_off + CH]
                                nc.vector.tensor_tensor(
                                    dst, u_seg[(blk, seg)], dst, AOP.add)
                            else:
                                import contextlib
                                hp = (tc.high_priority() if CONV_HIPRI
                                      else contextlib.nullcontext())
                                with hp:
                                    nc.gpsimd.dma_start(
                                        out=x13[:, r0:r0 + rows_per,
                                                x1_off:x1_off + CH],
                                        in_=u_seg[(blk, seg)].rearrange(
                                            "p (r c) -> p r c", c=CH),
                                        accum_op=AOP.add)
                    else:
                        ut = scratch.tile([128, CH], F16, tag=f"uv{blk}",
                                          name=f"uv{l}{di}{h}{tg}{blk}")
                        meng = MULT_PAT[mult_ctr[0] % len(MULT_PAT)]
                        mult_ctr[0] += 1
                        _eng(nc, meng).tensor_tensor(ut, g_c[blk], tt,
                                                     AOP.mult)
                        o = tg * x1_row_len + x1_off
                        nc.vector.tensor_tensor(x1[blk][:, o:o + CH], ut,
                                                r_dst[blk], AOP.add)
        def highway(l, di, h, x0, x1, x1_row_len, x1_off):
            u_seg = {}
            for tg in range(T // CH):
                highway_tg(l, di, h, tg, x0, x1, x1_row_len, x1_off, u_seg)

        # ---- network --------------------------------------------------------
        deferred_outs = []
        f_t = {}
        xa_t = {}
        for l in range(N_LAYERS):
            if l == 0:
                src = {0: (xpad0, ROW0, 0), 1: (xpad0, ROW0, W)}
            else:
                src = {0: (xpadf, ROW1, 0), 1: (xpadb, ROW1, 0)}
            for di in range(2):
                taps = fw[l] if di == 0 else bw[l]
                ft = [bufs.tile([128, T], F16, tag=f"f{di}{b}",
                                name=f"f{l}{di}{b}") for b in range(PB)]
                s_tiles, rl, off = src[di]
                for blk in range(PB):
                    mode = CONV_PAT[(l * 4 + di * 2 + blk) % len(CONV_PAT)]
                    import contextlib
                    hp = tc.high_priority() if CONV_HIPRI else contextlib.nullcontext()
                    with hp:
                        if mode == "dma":
                            conv_dma(l, di, blk, ft[blk], s_tiles[blk], rl,
                                     off, taps)
                        else:
                            conv(l, di, blk, ft[blk], s_tiles[blk], rl, off,
                                 taps)
                f_t[di] = ft

            if l == 0:
                xpadf = [bufs.tile([128, BLOC * ROW1], F16, tag=f"xpf{b}",
                                   name=f"xpf{b}") for b in range(PB)]
                xpadb = [bufs.tile([128, BLOC * ROW1], F16, tag=f"xpb{b}",
                                   name=f"xpb{b}") for b in range(PB)]
                for blk in range(PB):
                    for r in range(BLOC):
                        nc.vector.tensor_copy(
                            xpadf[blk][:, r * ROW1:r * ROW1 + W],
                            pad_sb[:, (0 * PB + blk) * W:(0 * PB + blk + 1) * W])
                        nc.vector.tensor_copy(
                            xpadb[blk][:, r * ROW1 + S:(r + 1) * ROW1],
                            pad_sb[:, (1 * PB + blk) * W:(1 * PB + blk + 1) * W])

            if INTERLEAVE_DIRS:
                for di in range(2):
                    xa_t[di] = [bufs.tile([128, T], F16, tag=f"xa{di}{b}",
                                          name=f"xa{l}{di}{b}")
                                for b in range(PB)]
                useg = {0: {}, 1: {}}
                for tg in range(T // CH):
                    for di in range(2):
                        highway_tg(l, di, 0, tg, f_t[di], xa_t[di], CH, 0,
                                   useg[di])
            else:
                for di in range(2):
                    xa = [bufs.tile([128, T], F16, tag=f"xa{di}{b}",
                                    name=f"xa{l}{di}{b}") for b in range(PB)]
                    highway(l, di, 0, f_t[di], xa, CH, 0)
                    xa_t[di] = xa

            x1s = {}
            for di in range(2):
                if l == 0:
                    x1s[di] = (xpadf if di == 0 else xpadb, ROW1,
                               (W if di == 0 else 0))
                else:
                    x1s[di] = ([bufs.tile([128, T], F16, tag=f"f{di}{b}",
                                          name=f"xb{l}{di}{b}")
                                for b in range(PB)], CH, 0)
            if INTERLEAVE_DIRS:
                useg = {0: {}, 1: {}}
                for tg in range(T // CH):
                    for di in range(2):
                        x1, rl, off = x1s[di]
                        highway_tg(l, di, 1, tg, xa_t[di], x1, rl, off,
                                   useg[di])
            else:
                for di in range(2):
                    x1, rl, off = x1s[di]
                    highway(l, di, 1, xa_t[di], x1, rl, off)
            for di in range(2):
                x1, rl, off = x1s[di]
                for blk in range(PB):
                    src_ap = x1[blk].rearrange("p (r c) -> p r c", c=rl)[:, :, off:off + CH]
                    dst_ap = out[l, di, blk].rearrange("p (r c) -> p r c", c=CH)
                    orows = 1 if (TAIL_FINE and l == N_LAYERS - 1) else OUT_ROWS
                    for s in range(BLOC // orows):
                        r0, r1 = s * orows, (s + 1) * orows
                        if DEFER_L0_OUT and l == 0:
                            deferred_outs.append((dst_ap[:, r0:r1],
                                                  src_ap[:, r0:r1]))
                        else:
                            nc.sync.dma_start(out=dst_ap[:, r0:r1],
                                              in_=src_ap[:, r0:r1])
            if l == N_LAYERS - 1:
                for dst_ap_, src_ap_ in deferred_outs:
                    nc.sync.dma_start(out=dst_ap_, in_=src_ap_)

        psum_cv.release()
        psum_hw.release()
        scratch.release()
        bufs.release()
        consts.release()

    nc.finalize()
    return nc


def _prep_params(inputs):
    fwd_hw_W = np.asarray(inputs["fwd_hw_W"], np.float32)
    bwd_hw_W = np.asarray(inputs["bwd_hw_W"], np.float32)
    wt = np.empty((N_LAYERS, 2, N_HW, PB, 128, 2 * D), np.float32)
    for l in range(N_LAYERS):
        for di, Wsrc in ((0, fwd_hw_W), (1, bwd_hw_W)):
            for h in range(N_HW):
                wT = Wsrc[l, h].T  # [D, 2D]
                wt[l, di, h] = wT.reshape(PB, 128, 2 * D)
    wt = wt.astype(np.float16)

    fwd_hw_b = np.asarray(inputs["fwd_hw_b"], np.float32)
    bwd_hw_b = np.asarray(inputs["bwd_hw_b"], np.float32)
    bias = np.empty((128, N_LAYERS * 2 * N_HW * EB), np.float32)
    for l in range(N_LAYERS):
        for di, bsrc in ((0, fwd_hw_b), (1, bwd_hw_b)):
            for h in range(N_HW):
                for eb in range(EB):
                    i = ((l * 2 + di) * N_HW + h) * EB + eb
                    bias[:, i] = bsrc[l, h, eb * 128:(eb + 1) * 128]

    fwd_pad = np.asarray(inputs["fwd_pad"], np.float32)
    bwd_pad = np.asarray(inputs["bwd_pad"], np.float32)
    pad1 = np.empty((128, 2 * PB * W), np.float32)
    for di, psrc in ((0, fwd_pad), (1, bwd_pad)):
        pT = psrc[1].T.reshape(PB, 128, W)
        for blk in range(PB):
            pad1[:, (di * PB + blk) * W:(di * PB + blk + 1) * W] = pT[blk]
    pad1 = pad1.astype(np.float16)

    fwd_w = np.asarray(inputs["fwd_w"], np.float32)
    bwd_w = np.asarray(inputs["bwd_w"], np.float32)
    diag = np.zeros((N_LAYERS, 2, 128, 5 * 128), np.float16)
    for l in range(N_LAYERS):
        for di, tw in ((0, fwd_w), (1, bwd_w)):
            for j in range(5):
                blkm = np.zeros((128, 128), np.float16)
                np.fill_diagonal(blkm, np.float16(tw[l, j]))
                diag[l, di, :, j * 128:(j + 1) * 128] = blkm

    return {
        "wt": np.ascontiguousarray(wt),
        "bias": np.ascontiguousarray(bias),
        "pad1": np.ascontiguousarray(pad1),
        "diag": np.ascontiguousarray(diag),
        "fwd_w": [[float(v) for v in row] for row in fwd_w],
        "bwd_w": [[float(v) for v in row] for row in bwd_w],
    }


def _prep_core_input(x_core, fwd_pad, bwd_pad):
    """x_core: [BLOC, S, D] f32 -> [PB, 128, BLOC*ROW0] f32 with halos."""
    xt = np.ascontiguousarray(x_core.transpose(2, 0, 1))  # [D, BLOC, S]
    blocks = xt.reshape(PB, 128, BLOC, S)
    padded = np.empty((PB, 128, BLOC, ROW0), np.float32)
    padded[:, :, :, W:W + S] = blocks
    fr = fwd_pad[0].T.reshape(PB, 128, W)
    bk = bwd_pad[0].T.reshape(PB, 128, W)
    padded[:, :, :, :W] = fr[:, :, None, :]
    padded[:, :, :, W + S:] = bk[:, :, None, :]
    return np.ascontiguousarray(padded.reshape(PB, 128, BLOC * ROW0))


_NC_CACHE = {}


def kernel(**inputs):
    params = _prep_params(inputs)
    import hashlib
    h = hashlib.sha256()
    for k in ("wt", "bias", "pad1", "diag"):
        h.update(params[k].tobytes())
    h.update(repr(params["fwd_w"]).encode())
    h.update(repr(params["bwd_w"]).encode())
    key = h.hexdigest()
    if key not in _NC_CACHE:
        _NC_CACHE[key] = build_bass(params)
    nc = _NC_CACHE[key]

    x = np.asarray(inputs["inputs"], np.float32)
    fwd_pad = np.asarray(inputs["fwd_pad"], np.float32)
    bwd_pad = np.asarray(inputs["bwd_pad"], np.float32)
    in_maps = [
        {"x": _prep_core_input(x[c * BLOC:(c + 1) * BLOC], fwd_pad, bwd_pad)}
        for c in range(NCORES)
    ]
    res = run_bass_kernel_spmd(nc, in_maps, core_ids=list(range(NCORES)))

    y = np.empty((N_LAYERS, B, S, 2 * D), np.float32)
    for c in range(NCORES):
        o = np.asarray(res.results[c]["out"]).astype(np.float32)
        o = o.reshape(N_LAYERS, 2, PB, 128, BLOC, S)
        o = o.transpose(0, 4, 5, 1, 2, 3).reshape(N_LAYERS, BLOC, S, 2 * D)
        y[:, c * BLOC:(c + 1) * BLOC] = o
    return y

